# revision 17
# baseline (speedup 1.0000x reference)
"""Trainium2 Bass kernel for the GCNN message-passing module.

Strategy (8-way data/graph parallel, nodes sharded by destination):
  - Each core owns a contiguous block of 2560 destination nodes (N padded
    20000 -> 20480). Full node features + relation weights are replicated
    to every core's HBM.
  - Messages (2 per edge: head<-W_r(tail), tail<-W_{r+R}(head)) are
    partitioned by destination on the host, sorted by (relation, dest),
    and packed into 128-message chunks per (relation, 512-dest window).
  - On device, per (relation, window): indirect-DMA gather of the source
    rows (f32 HBM -> bf16 SBUF cast in flight), then one PE matmul per
    chunk with a one-hot assignment matrix A[msg, dest] built on-chip:
    G^T[feat, dest] += X_chunk^T @ A  -- the PE does the transpose AND the
    segment-sum in a single pass, accumulating in PSUM.
  - Dense transform: agg^T[:, win] = sum_w W_w @ G_w^T + W_self @ x^T
    + bias * counts, all as bf16 matmuls accumulating in f32 PSUM.
  - PE-transpose agg^T back to [dest, feat], fused ReLU, DMA out.
  - Host concatenates the 8 disjoint row shards.
"""

import os
import sys
from dataclasses import dataclass

sys.path.insert(0, "/opt/trn_rl_repo")

import numpy as np
import ml_dtypes

import concourse.bass as bass
import concourse.bacc as bacc
import concourse.tile as tile
from concourse import bass_utils, mybir
from concourse.bass import ds, ts
BF16 = ml_dtypes.bfloat16
NCORES = 8
R = 8
NW = 2 * R          # 16 relation weights
D = 512
KB = D // 128       # feature blocks
P = 128
SW = 512            # dest super-window (one PSUM bank of f32)

LAST_RESULT = None  # BassKernelResults of the last kernel() call (for test.py)


@dataclass
class Cfg:
    N: int          # true number of nodes
    NPAD: int       # padded to NCORES * NSW * SW
    CORE_NODES: int
    NSW: int


def make_cfg(n_nodes: int) -> Cfg:
    per_core = -(-n_nodes // NCORES)
    nsw = -(-per_core // SW)
    core_nodes = nsw * SW
    return Cfg(N=n_nodes, NPAD=core_nodes * NCORES, CORE_NODES=core_nodes, NSW=nsw)


def _host_prep(cfg, inp, heads, tails, rel, W_self, b_self, W_rel, b_rel):
    """Build per-core input tensors + the (uniform across cores) chunk plan."""
    NSW, CORE_NODES = cfg.NSW, cfg.CORE_NODES
    NKEY = NW * NSW

    dest = np.concatenate([heads, tails]).astype(np.int64)
    srcs = np.concatenate([tails, heads]).astype(np.int64)
    wgt = np.concatenate([rel, rel + R]).astype(np.int64)

    core = dest // CORE_NODES
    percore = []
    for c in range(NCORES):
        m = core == c
        dl = dest[m] - c * CORE_NODES
        s = srcs[m]
        w = wgt[m]
        order = np.lexsort((dl, w))
        dl, s, w = dl[order], s[order], w[order]
        key = w * NSW + dl // SW
        cnts = np.bincount(key, minlength=NKEY)
        percore.append((dl, s, w, key, cnts))

    allcnts = np.stack([p[4] for p in percore])                  # [8, NKEY]
    nch = np.maximum(-(-allcnts.max(0) // P), 1).astype(np.int64)  # chunks/key
    chunk_base = np.concatenate([[0], np.cumsum(nch)])
    NCHUNK = int(chunk_base[-1])

    idx_all = np.zeros((NCORES, P, NCHUNK), np.int32)
    dr_all = np.full((NCORES, P, NCHUNK), -1.0, np.float32)
    cnt_all = np.zeros((NCORES, 32, CORE_NODES), np.float32)
    # per-chunk dest-window lo/hi across all cores (window-relative dest)
    wlo = np.full(NCHUNK, SW, np.int64)
    whi = np.full(NCHUNK, -1, np.int64)
    for c in range(NCORES):
        dl, s, w, key, cnts = percore[c]
        koff = np.concatenate([[0], np.cumsum(cnts)])
        pos = np.arange(len(dl)) - koff[key]
        col = chunk_base[key] + pos // P
        row = pos % P
        idx_all[c, row, col] = s
        drel = dl - (key % NSW) * SW
        dr_all[c, row, col] = drel
        np.minimum.at(wlo, col, drel)
        np.maximum.at(whi, col, drel)
        np.add.at(cnt_all[c], (w, dl), 1.0)
        cnt_all[c][16, :] = 1.0
        # sort each chunk's rows by source id: SDMA random reads become
        # ascending-address reads (the A matrix absorbs the permutation)
        order = np.argsort(idx_all[c], axis=0, kind="stable")
        idx_all[c] = np.take_along_axis(idx_all[c], order, 0)
        dr_all[c] = np.take_along_axis(dr_all[c], order, 0)

    # chunk plan: (w, sw, ci_base, n_chunks, [(a, wn) per chunk]).
    # Windows tile [0, SW) contiguously: chunk j covers [s_j, e_j) with
    # s_0 = 0, s_j = min-core first-dest, e_j = max(max-core last-dest + 1,
    # s_{j+1}), e_last = SW. Every PSUM column is written by >=1 chunk, so
    # only chunk 0 needs start=True (clears the bank's has_written bits);
    # later chunks overwrite-where-unset / accumulate-where-set.
    plan = []
    for key in range(NKEY):
        cb, n = int(chunk_base[key]), int(nch[key])
        starts = [0] + [int(wlo[cb + j]) for j in range(1, n)]
        ends = [
            max(int(whi[cb + j]) + 1, starts[j + 1]) if j < n - 1 else SW
            for j in range(n)
        ]
        wins = []
        for j in range(n):
            a, wn = starts[j], ends[j] - starts[j]
            assert wn >= 1
            wins.append((a, wn))
            # rebase destrel to the window start (pads stay negative)
            for c in range(NCORES):
                real = dr_all[c, :, cb + j] >= 0
                dr_all[c, real, cb + j] -= a
        plan.append((key // NSW, key % NSW, cb, n, wins))

    inp_pad = np.zeros((cfg.NPAD, D), np.float32)
    inp_pad[: cfg.N] = inp

    # W^T packed [p, 17, kb, o]: slice [:, w, kb, ob*128:(ob+1)*128] is the
    # [K=feat-block, M=out-block] stationary operand.
    Wall = np.concatenate([W_rel, W_self[None]], 0)              # [17, o, in]
    wt = np.ascontiguousarray(
        Wall.transpose(2, 0, 1).reshape(KB, P, 17, D).transpose(1, 2, 0, 3)
    ).astype(BF16)                                               # [p,17,kb,o]

    baug = np.zeros((32, D), np.float32)
    baug[:NW] = b_rel
    baug[16] = b_self
    baug = baug.astype(BF16)

    iota = np.tile(np.arange(SW, dtype=np.float32), (P, 1))

    in_maps = []
    for c in range(NCORES):
        sl = inp_pad[c * CORE_NODES : (c + 1) * CORE_NODES]
        ipt = np.ascontiguousarray(
            sl.T.reshape(KB, P, CORE_NODES).transpose(1, 0, 2)
        ).astype(BF16)                                           # [p, kb, j]
        in_maps.append(
            {
                "inp": inp_pad.astype(BF16),
                "idx": np.ascontiguousarray(idx_all[c]),
                "dr": np.ascontiguousarray(dr_all[c]),
                "iota": iota,
                "ipt": ipt,
                "wt": wt,
                "cnt": cnt_all[c].astype(BF16),
                "baug": baug,
            }
        )
    return in_maps, plan, NCHUNK, int(nch.max())


def _emit(tc, out_ap, ins, cfg, plan, NCHUNK, NCHMAX):
    nc = tc.nc
    f32 = mybir.dt.float32
    bf16 = mybir.dt.bfloat16
    i32 = mybir.dt.int32
    NSW, CORE_NODES = cfg.NSW, cfg.CORE_NODES

    # plan indexed by (w, sw)
    bykey = {}
    for w, sw, cb, n, wins in plan:
        bykey[(w, sw)] = (cb, n, wins)

    with (
        tc.tile_pool(name="const", bufs=1) as const,
        tc.tile_pool(name="xp", bufs=16) as xp,
        tc.tile_pool(name="apl", bufs=10) as apl,
        tc.tile_pool(name="gsb", bufs=5) as gsb,
        tc.tile_pool(name="asb", bufs=2) as asb,
        tc.tile_pool(name="osb", bufs=6) as osb,
        tc.tile_pool(name="gps", bufs=4, space="PSUM") as gps,
        tc.tile_pool(name="aps", bufs=4, space="PSUM") as aps,
    ):
        idx_sb = const.tile([P, NCHUNK], i32)
        nc.sync.dma_start(idx_sb[:], ins["idx"][:, :])
        dr_sb = const.tile([P, NCHUNK], f32)
        nc.sync.dma_start(dr_sb[:], ins["dr"][:, :])
        iota_sb = const.tile([P, SW], f32)
        nc.sync.dma_start(iota_sb[:], ins["iota"][:, :])
        wt_sb = const.tile([P, 17, KB, D], bf16)
        nc.sync.dma_start(wt_sb[:], ins["wt"][:, :, :, :])
        ipt_sb = const.tile([P, KB, CORE_NODES], bf16)
        nc.sync.dma_start(ipt_sb[:], ins["ipt"][:, :, :])
        cnt_sb = const.tile([32, CORE_NODES], bf16)
        nc.sync.dma_start(cnt_sb[:], ins["cnt"][:, :])
        baug_sb = const.tile([32, D], bf16)
        nc.sync.dma_start(baug_sb[:], ins["baug"][:, :])
        inp_dram = ins["inp"]

        for swi in range(NSW):
            # --- agg accumulation, one PSUM bank per 128-dest block:
            # out[dest, feat] directly (G/x/cnt as stationary). bias + self first.
            apt = [aps.tile([P, D], f32, tag="ps", name=f"apt{_db}") for _db in range(KB)]
            for db in range(KB):
                nc.tensor.matmul(
                    apt[db][:],
                    lhsT=cnt_sb[:, ds(swi * SW + db * P, P)],
                    rhs=baug_sb[:, :],
                    start=True,
                    stop=False,
                )
            for kb in range(KB):
                for db in range(KB):
                    nc.tensor.matmul(
                        apt[db][:],
                        lhsT=ipt_sb[:, kb, ds(swi * SW + db * P, P)],
                        rhs=wt_sb[:, 16, kb, :],
                        start=False,
                        stop=False,
                    )

            # software pipeline: agg matmuls run 2 groups behind step2, so the
            # PSUM->SBUF evacuation (DVE) never stalls the in-order PE.
            DELAY = 2
            gts = {}
            for wi in range(NW + DELAY):
                if wi < NW:
                    w = wi
                    cb, n, wins = bykey[(w, swi)]
                    gpt = [gps.tile([P, SW], f32, tag="gp", name=f"gpt{_mb}") for _mb in range(KB)]
                    xts = []
                    for j in range(n):
                        xt = xp.tile([P, D], bf16, tag="x", name=f"x{j}")
                        nc.gpsimd.indirect_dma_start(
                            out=xt[:],
                            out_offset=None,
                            in_=inp_dram[:, :],
                            in_offset=bass.IndirectOffsetOnAxis(
                                ap=idx_sb[:, cb + j : cb + j + 1], axis=0
                            ),
                        )
                        xts.append(xt)
                    for j in range(n):
                        a, wn = wins[j]
                        at = apl.tile([P, SW], bf16, tag="a")
                        nc.vector.tensor_scalar(
                            at[:, :wn],
                            iota_sb[:, :wn],
                            dr_sb[:, cb + j : cb + j + 1],
                            None,
                            mybir.AluOpType.is_equal,
                        )
                        for mb in range(KB):
                            nc.tensor.matmul(
                                gpt[mb][:, a : a + wn],
                                lhsT=xts[j][:, ts(mb, P)],
                                rhs=at[:, :wn],
                                start=(j == 0),
                                stop=(j == n - 1),
                            )
                    gt = gsb.tile([P, KB, SW], bf16)
                    for mb in range(KB):
                        nc.vector.tensor_copy(gt[:, mb, :], gpt[mb][:])
                    gts[w] = gt
                if wi >= DELAY:
                    w = wi - DELAY
                    gt = gts.pop(w)
                    for kb in range(KB):
                        for db in range(KB):
                            nc.tensor.matmul(
                                apt[db][:],
                                lhsT=gt[:, kb, ts(db, P)],
                                rhs=wt_sb[:, w, kb, :],
                                start=False,
                                stop=(w == NW - 1 and kb == KB - 1),
                            )

            # --- relu + store (already [dest, feat])
            for db in range(KB):
                ot = osb.tile([P, D], f32)
                nc.vector.tensor_scalar(
                    ot[:], apt[db][:], 0.0, None, mybir.AluOpType.max
                )
                nc.sync.dma_start(out_ap[ds(swi * SW + db * P, P), :], ot[:])


def _build(cfg, plan, NCHUNK, NCHMAX):
    nc = bacc.Bacc("TRN2", target_bir_lowering=False, debug=False,
                   num_devices=NCORES)
    f32 = mybir.dt.float32
    ins = {
        "inp": nc.dram_tensor("inp", (cfg.NPAD, D), mybir.dt.bfloat16, kind="ExternalInput").ap(),
        "idx": nc.dram_tensor("idx", (P, NCHUNK), mybir.dt.int32, kind="ExternalInput").ap(),
        "dr": nc.dram_tensor("dr", (P, NCHUNK), f32, kind="ExternalInput").ap(),
        "iota": nc.dram_tensor("iota", (P, SW), f32, kind="ExternalInput").ap(),
        "ipt": nc.dram_tensor("ipt", (P, KB, cfg.CORE_NODES), mybir.dt.bfloat16, kind="ExternalInput").ap(),
        "wt": nc.dram_tensor("wt", (P, 17, KB, D), mybir.dt.bfloat16, kind="ExternalInput").ap(),
        "cnt": nc.dram_tensor("cnt", (32, cfg.CORE_NODES), mybir.dt.bfloat16, kind="ExternalInput").ap(),
        "baug": nc.dram_tensor("baug", (32, D), mybir.dt.bfloat16, kind="ExternalInput").ap(),
    }
    out = nc.dram_tensor("out", (cfg.CORE_NODES, D), f32, kind="ExternalOutput").ap()
    with tile.TileContext(nc) as tc:
        _emit(tc, out, ins, cfg, plan, NCHUNK, NCHMAX)
    nc.compile()
    return nc


def kernel(**inputs):
    global LAST_RESULT
    a = {k: np.asarray(v) for k, v in inputs.items()}
    inp = a["input"].astype(np.float32)
    cfg = make_cfg(inp.shape[0])
    in_maps, plan, NCHUNK, NCHMAX = _host_prep(
        cfg, inp, a["heads"], a["tails"], a["rel"],
        a["W_self"].astype(np.float32), a["b_self"].astype(np.float32),
        a["W_rel"].astype(np.float32), a["b_rel"].astype(np.float32),
    )
    nc = _build(cfg, plan, NCHUNK, NCHMAX)
    res = bass_utils.run_bass_kernel_spmd(
        nc, in_maps, core_ids=list(range(NCORES)),
        trace=bool(os.environ.get("KERNEL_TRACE")),
    )
    LAST_RESULT = res
    full = np.concatenate([res.results[c]["out"] for c in range(NCORES)], 0)
    return full[: cfg.N].astype(np.float32)


# revision 18
# speedup vs baseline: 1.0065x; 1.0065x over previous
"""Trainium2 Bass kernel for the GCNN message-passing module.

Strategy (8-way data/graph parallel, nodes sharded by destination):
  - Each core owns a contiguous block of 2560 destination nodes (N padded
    20000 -> 20480). Full node features + relation weights are replicated
    to every core's HBM.
  - Messages (2 per edge: head<-W_r(tail), tail<-W_{r+R}(head)) are
    partitioned by destination on the host, sorted by (relation, dest),
    and packed into 128-message chunks per (relation, 512-dest window).
  - On device, per (relation, window): indirect-DMA gather of the source
    rows (f32 HBM -> bf16 SBUF cast in flight), then one PE matmul per
    chunk with a one-hot assignment matrix A[msg, dest] built on-chip:
    G^T[feat, dest] += X_chunk^T @ A  -- the PE does the transpose AND the
    segment-sum in a single pass, accumulating in PSUM.
  - Dense transform: agg^T[:, win] = sum_w W_w @ G_w^T + W_self @ x^T
    + bias * counts, all as bf16 matmuls accumulating in f32 PSUM.
  - PE-transpose agg^T back to [dest, feat], fused ReLU, DMA out.
  - Host concatenates the 8 disjoint row shards.
"""

import os
import sys
from dataclasses import dataclass

sys.path.insert(0, "/opt/trn_rl_repo")

import numpy as np
import ml_dtypes

import concourse.bass as bass
import concourse.bacc as bacc
import concourse.tile as tile
from concourse import bass_utils, mybir
from concourse.bass import ds, ts
BF16 = ml_dtypes.bfloat16
NCORES = 8
R = 8
NW = 2 * R          # 16 relation weights
D = 512
KB = D // 128       # feature blocks
P = 128
SW = 512            # dest super-window (one PSUM bank of f32)

LAST_RESULT = None  # BassKernelResults of the last kernel() call (for test.py)


@dataclass
class Cfg:
    N: int          # true number of nodes
    NPAD: int       # padded to NCORES * NSW * SW
    CORE_NODES: int
    NSW: int


def make_cfg(n_nodes: int) -> Cfg:
    per_core = -(-n_nodes // NCORES)
    nsw = -(-per_core // SW)
    core_nodes = nsw * SW
    return Cfg(N=n_nodes, NPAD=core_nodes * NCORES, CORE_NODES=core_nodes, NSW=nsw)


def _host_prep(cfg, inp, heads, tails, rel, W_self, b_self, W_rel, b_rel):
    """Build per-core input tensors + the (uniform across cores) chunk plan."""
    NSW, CORE_NODES = cfg.NSW, cfg.CORE_NODES
    NKEY = NW * NSW

    dest = np.concatenate([heads, tails]).astype(np.int64)
    srcs = np.concatenate([tails, heads]).astype(np.int64)
    wgt = np.concatenate([rel, rel + R]).astype(np.int64)

    core = dest // CORE_NODES
    percore = []
    for c in range(NCORES):
        m = core == c
        dl = dest[m] - c * CORE_NODES
        s = srcs[m]
        w = wgt[m]
        order = np.lexsort((dl, w))
        dl, s, w = dl[order], s[order], w[order]
        key = w * NSW + dl // SW
        cnts = np.bincount(key, minlength=NKEY)
        percore.append((dl, s, w, key, cnts))

    allcnts = np.stack([p[4] for p in percore])                  # [8, NKEY]
    nch = np.maximum(-(-allcnts.max(0) // P), 1).astype(np.int64)  # chunks/key
    chunk_base = np.concatenate([[0], np.cumsum(nch)])
    NCHUNK = int(chunk_base[-1])

    idx_all = np.zeros((NCORES, P, NCHUNK), np.int32)
    dr_all = np.full((NCORES, P, NCHUNK), -1.0, np.float32)
    cnt_all = np.zeros((NCORES, 32, CORE_NODES), np.float32)
    # per-chunk dest-window lo/hi across all cores (window-relative dest)
    wlo = np.full(NCHUNK, SW, np.int64)
    whi = np.full(NCHUNK, -1, np.int64)
    for c in range(NCORES):
        dl, s, w, key, cnts = percore[c]
        koff = np.concatenate([[0], np.cumsum(cnts)])
        pos = np.arange(len(dl)) - koff[key]
        col = chunk_base[key] + pos // P
        row = pos % P
        idx_all[c, row, col] = s
        drel = dl - (key % NSW) * SW
        dr_all[c, row, col] = drel
        np.minimum.at(wlo, col, drel)
        np.maximum.at(whi, col, drel)
        np.add.at(cnt_all[c], (w, dl), 1.0)
        cnt_all[c][16, :] = 1.0
        # sort each chunk's rows by source id: SDMA random reads become
        # ascending-address reads (the A matrix absorbs the permutation)
        order = np.argsort(idx_all[c], axis=0, kind="stable")
        idx_all[c] = np.take_along_axis(idx_all[c], order, 0)
        dr_all[c] = np.take_along_axis(dr_all[c], order, 0)

    # chunk plan: (w, sw, ci_base, n_chunks, [(a, wn) per chunk]).
    # Windows tile [0, SW) contiguously: chunk j covers [s_j, e_j) with
    # s_0 = 0, s_j = min-core first-dest, e_j = max(max-core last-dest + 1,
    # s_{j+1}), e_last = SW. Every PSUM column is written by >=1 chunk, so
    # only chunk 0 needs start=True (clears the bank's has_written bits);
    # later chunks overwrite-where-unset / accumulate-where-set.
    plan = []
    for key in range(NKEY):
        cb, n = int(chunk_base[key]), int(nch[key])
        starts = [0] + [int(wlo[cb + j]) for j in range(1, n)]
        ends = [
            max(int(whi[cb + j]) + 1, starts[j + 1]) if j < n - 1 else SW
            for j in range(n)
        ]
        wins = []
        for j in range(n):
            a, wn = starts[j], ends[j] - starts[j]
            assert wn >= 1
            wins.append((a, wn))
            # rebase destrel to the window start (pads stay negative)
            for c in range(NCORES):
                real = dr_all[c, :, cb + j] >= 0
                dr_all[c, real, cb + j] -= a
        plan.append((key // NSW, key % NSW, cb, n, wins))

    inp_pad = np.zeros((cfg.NPAD, D), np.float32)
    inp_pad[: cfg.N] = inp

    # W^T packed [p, 17, kb, o]: slice [:, w, kb, ob*128:(ob+1)*128] is the
    # [K=feat-block, M=out-block] stationary operand.
    Wall = np.concatenate([W_rel, W_self[None]], 0)              # [17, o, in]
    wt = np.ascontiguousarray(
        Wall.transpose(2, 0, 1).reshape(KB, P, 17, D).transpose(1, 2, 0, 3)
    ).astype(BF16)                                               # [p,17,kb,o]

    baug = np.zeros((32, D), np.float32)
    baug[:NW] = b_rel
    baug[16] = b_self
    baug = baug.astype(BF16)

    iota = np.tile(np.arange(SW, dtype=np.float32), (P, 1))

    in_maps = []
    for c in range(NCORES):
        sl = inp_pad[c * CORE_NODES : (c + 1) * CORE_NODES]
        ipt = np.ascontiguousarray(
            sl.T.reshape(KB, P, CORE_NODES).transpose(1, 0, 2)
        ).astype(BF16)                                           # [p, kb, j]
        in_maps.append(
            {
                "inp": inp_pad.astype(BF16),
                "idx": np.ascontiguousarray(idx_all[c]),
                "dr": np.ascontiguousarray(dr_all[c]),
                "iota": iota,
                "ipt": ipt,
                "wt": wt,
                "cnt": cnt_all[c].astype(BF16),
                "baug": baug,
            }
        )
    return in_maps, plan, NCHUNK, int(nch.max())


def _emit(tc, out_ap, ins, cfg, plan, NCHUNK, NCHMAX):
    nc = tc.nc
    f32 = mybir.dt.float32
    bf16 = mybir.dt.bfloat16
    i32 = mybir.dt.int32
    NSW, CORE_NODES = cfg.NSW, cfg.CORE_NODES

    # plan indexed by (w, sw)
    bykey = {}
    for w, sw, cb, n, wins in plan:
        bykey[(w, sw)] = (cb, n, wins)

    with (
        tc.tile_pool(name="const", bufs=1) as const,
        tc.tile_pool(name="xp", bufs=24) as xp,
        tc.tile_pool(name="apl", bufs=10) as apl,
        tc.tile_pool(name="gsb", bufs=5) as gsb,
        tc.tile_pool(name="asb", bufs=2) as asb,
        tc.tile_pool(name="osb", bufs=6) as osb,
        tc.tile_pool(name="gps", bufs=4, space="PSUM") as gps,
        tc.tile_pool(name="aps", bufs=4, space="PSUM") as aps,
    ):
        idx_sb = const.tile([P, NCHUNK], i32)
        nc.sync.dma_start(idx_sb[:], ins["idx"][:, :])
        dr_sb = const.tile([P, NCHUNK], f32)
        nc.sync.dma_start(dr_sb[:], ins["dr"][:, :])
        iota_sb = const.tile([P, SW], f32)
        nc.sync.dma_start(iota_sb[:], ins["iota"][:, :])
        wt_sb = const.tile([P, 17, KB, D], bf16)
        nc.sync.dma_start(wt_sb[:], ins["wt"][:, :, :, :])
        ipt_sb = const.tile([P, KB, CORE_NODES], bf16)
        nc.sync.dma_start(ipt_sb[:], ins["ipt"][:, :, :])
        cnt_sb = const.tile([32, CORE_NODES], bf16)
        nc.sync.dma_start(cnt_sb[:], ins["cnt"][:, :])
        baug_sb = const.tile([32, D], bf16)
        nc.sync.dma_start(baug_sb[:], ins["baug"][:, :])
        inp_dram = ins["inp"]

        for swi in range(NSW):
            # --- agg accumulation, one PSUM bank per 128-dest block:
            # out[dest, feat] directly (G/x/cnt as stationary). bias + self first.
            apt = [aps.tile([P, D], f32, tag="ps", name=f"apt{_db}") for _db in range(KB)]
            for db in range(KB):
                nc.tensor.matmul(
                    apt[db][:],
                    lhsT=cnt_sb[:, ds(swi * SW + db * P, P)],
                    rhs=baug_sb[:, :],
                    start=True,
                    stop=False,
                )
            for kb in range(KB):
                for db in range(KB):
                    nc.tensor.matmul(
                        apt[db][:],
                        lhsT=ipt_sb[:, kb, ds(swi * SW + db * P, P)],
                        rhs=wt_sb[:, 16, kb, :],
                        start=False,
                        stop=False,
                    )

            # software pipeline: agg matmuls run 2 groups behind step2, so the
            # PSUM->SBUF evacuation (DVE) never stalls the in-order PE.
            DELAY = 2
            gts = {}
            for wi in range(NW + DELAY):
                if wi < NW:
                    w = wi
                    cb, n, wins = bykey[(w, swi)]
                    gpt = [gps.tile([P, SW], f32, tag="gp", name=f"gpt{_mb}") for _mb in range(KB)]
                    xts = []
                    for j in range(n):
                        xt = xp.tile([P, D], bf16, tag="x", name=f"x{j}")
                        nc.gpsimd.indirect_dma_start(
                            out=xt[:],
                            out_offset=None,
                            in_=inp_dram[:, :],
                            in_offset=bass.IndirectOffsetOnAxis(
                                ap=idx_sb[:, cb + j : cb + j + 1], axis=0
                            ),
                        )
                        xts.append(xt)
                    for j in range(n):
                        a, wn = wins[j]
                        at = apl.tile([P, SW], bf16, tag="a")
                        nc.vector.tensor_scalar(
                            at[:, :wn],
                            iota_sb[:, :wn],
                            dr_sb[:, cb + j : cb + j + 1],
                            None,
                            mybir.AluOpType.is_equal,
                        )
                        for mb in range(KB):
                            nc.tensor.matmul(
                                gpt[mb][:, a : a + wn],
                                lhsT=xts[j][:, ts(mb, P)],
                                rhs=at[:, :wn],
                                start=(j == 0),
                                stop=(j == n - 1),
                            )
                    gt = gsb.tile([P, KB, SW], bf16)
                    for mb in range(KB):
                        nc.vector.tensor_copy(gt[:, mb, :], gpt[mb][:])
                    gts[w] = gt
                if wi >= DELAY:
                    w = wi - DELAY
                    gt = gts.pop(w)
                    for kb in range(KB):
                        for db in range(KB):
                            nc.tensor.matmul(
                                apt[db][:],
                                lhsT=gt[:, kb, ts(db, P)],
                                rhs=wt_sb[:, w, kb, :],
                                start=False,
                                stop=(w == NW - 1 and kb == KB - 1),
                            )

            # --- relu + store (already [dest, feat])
            for db in range(KB):
                ot = osb.tile([P, D], f32)
                nc.vector.tensor_scalar(
                    ot[:], apt[db][:], 0.0, None, mybir.AluOpType.max
                )
                nc.sync.dma_start(out_ap[ds(swi * SW + db * P, P), :], ot[:])


def _build(cfg, plan, NCHUNK, NCHMAX):
    nc = bacc.Bacc("TRN2", target_bir_lowering=False, debug=False,
                   num_devices=NCORES, dynamic_dma_scratch_size=32768)
    f32 = mybir.dt.float32
    ins = {
        "inp": nc.dram_tensor("inp", (cfg.NPAD, D), mybir.dt.bfloat16, kind="ExternalInput").ap(),
        "idx": nc.dram_tensor("idx", (P, NCHUNK), mybir.dt.int32, kind="ExternalInput").ap(),
        "dr": nc.dram_tensor("dr", (P, NCHUNK), f32, kind="ExternalInput").ap(),
        "iota": nc.dram_tensor("iota", (P, SW), f32, kind="ExternalInput").ap(),
        "ipt": nc.dram_tensor("ipt", (P, KB, cfg.CORE_NODES), mybir.dt.bfloat16, kind="ExternalInput").ap(),
        "wt": nc.dram_tensor("wt", (P, 17, KB, D), mybir.dt.bfloat16, kind="ExternalInput").ap(),
        "cnt": nc.dram_tensor("cnt", (32, cfg.CORE_NODES), mybir.dt.bfloat16, kind="ExternalInput").ap(),
        "baug": nc.dram_tensor("baug", (32, D), mybir.dt.bfloat16, kind="ExternalInput").ap(),
    }
    out = nc.dram_tensor("out", (cfg.CORE_NODES, D), f32, kind="ExternalOutput").ap()
    with tile.TileContext(nc) as tc:
        _emit(tc, out, ins, cfg, plan, NCHUNK, NCHMAX)
    nc.compile()
    return nc


def kernel(**inputs):
    global LAST_RESULT
    a = {k: np.asarray(v) for k, v in inputs.items()}
    inp = a["input"].astype(np.float32)
    cfg = make_cfg(inp.shape[0])
    in_maps, plan, NCHUNK, NCHMAX = _host_prep(
        cfg, inp, a["heads"], a["tails"], a["rel"],
        a["W_self"].astype(np.float32), a["b_self"].astype(np.float32),
        a["W_rel"].astype(np.float32), a["b_rel"].astype(np.float32),
    )
    nc = _build(cfg, plan, NCHUNK, NCHMAX)
    res = bass_utils.run_bass_kernel_spmd(
        nc, in_maps, core_ids=list(range(NCORES)),
        trace=bool(os.environ.get("KERNEL_TRACE")),
    )
    LAST_RESULT = res
    full = np.concatenate([res.results[c]["out"] for c in range(NCORES)], 0)
    return full[: cfg.N].astype(np.float32)


# revision 19
# speedup vs baseline: 1.0105x; 1.0040x over previous
"""Trainium2 Bass kernel for the GCNN message-passing module.

Strategy (8-way data/graph parallel, nodes sharded by destination):
  - Each core owns a contiguous block of 2560 destination nodes (N padded
    20000 -> 20480). Full node features + relation weights are replicated
    to every core's HBM.
  - Messages (2 per edge: head<-W_r(tail), tail<-W_{r+R}(head)) are
    partitioned by destination on the host, sorted by (relation, dest),
    and packed into 128-message chunks per (relation, 512-dest window).
  - On device, per (relation, window): indirect-DMA gather of the source
    rows (f32 HBM -> bf16 SBUF cast in flight), then one PE matmul per
    chunk with a one-hot assignment matrix A[msg, dest] built on-chip:
    G^T[feat, dest] += X_chunk^T @ A  -- the PE does the transpose AND the
    segment-sum in a single pass, accumulating in PSUM.
  - Dense transform: agg^T[:, win] = sum_w W_w @ G_w^T + W_self @ x^T
    + bias * counts, all as bf16 matmuls accumulating in f32 PSUM.
  - PE-transpose agg^T back to [dest, feat], fused ReLU, DMA out.
  - Host concatenates the 8 disjoint row shards.
"""

import os
import sys
from dataclasses import dataclass

sys.path.insert(0, "/opt/trn_rl_repo")

import numpy as np
import ml_dtypes

import concourse.bass as bass
import concourse.bacc as bacc
import concourse.tile as tile
from concourse import bass_utils, mybir
from concourse.bass import ds, ts
BF16 = ml_dtypes.bfloat16
NCORES = 8
R = 8
NW = 2 * R          # 16 relation weights
D = 512
KB = D // 128       # feature blocks
P = 128
SW = 512            # dest super-window (one PSUM bank of f32)

LAST_RESULT = None  # BassKernelResults of the last kernel() call (for test.py)


@dataclass
class Cfg:
    N: int          # true number of nodes
    NPAD: int       # padded to NCORES * NSW * SW
    CORE_NODES: int
    NSW: int


def make_cfg(n_nodes: int) -> Cfg:
    per_core = -(-n_nodes // NCORES)
    nsw = -(-per_core // SW)
    core_nodes = nsw * SW
    return Cfg(N=n_nodes, NPAD=core_nodes * NCORES, CORE_NODES=core_nodes, NSW=nsw)


def _host_prep(cfg, inp, heads, tails, rel, W_self, b_self, W_rel, b_rel):
    """Build per-core input tensors + the (uniform across cores) chunk plan."""
    NSW, CORE_NODES = cfg.NSW, cfg.CORE_NODES
    NKEY = NW * NSW

    dest = np.concatenate([heads, tails]).astype(np.int64)
    srcs = np.concatenate([tails, heads]).astype(np.int64)
    wgt = np.concatenate([rel, rel + R]).astype(np.int64)

    core = dest // CORE_NODES
    percore = []
    for c in range(NCORES):
        m = core == c
        dl = dest[m] - c * CORE_NODES
        s = srcs[m]
        w = wgt[m]
        order = np.lexsort((dl, w))
        dl, s, w = dl[order], s[order], w[order]
        key = w * NSW + dl // SW
        cnts = np.bincount(key, minlength=NKEY)
        percore.append((dl, s, w, key, cnts))

    allcnts = np.stack([p[4] for p in percore])                  # [8, NKEY]
    nch = np.maximum(-(-allcnts.max(0) // P), 1).astype(np.int64)  # chunks/key
    chunk_base = np.concatenate([[0], np.cumsum(nch)])
    NCHUNK = int(chunk_base[-1])

    idx_all = np.zeros((NCORES, P, NCHUNK), np.int32)
    dr_all = np.full((NCORES, P, NCHUNK), -1.0, np.float32)
    cnt_all = np.zeros((NCORES, 32, CORE_NODES), np.float32)
    # per-chunk dest-window lo/hi across all cores (window-relative dest)
    wlo = np.full(NCHUNK, SW, np.int64)
    whi = np.full(NCHUNK, -1, np.int64)
    for c in range(NCORES):
        dl, s, w, key, cnts = percore[c]
        koff = np.concatenate([[0], np.cumsum(cnts)])
        pos = np.arange(len(dl)) - koff[key]
        col = chunk_base[key] + pos // P
        row = pos % P
        idx_all[c, row, col] = s
        drel = dl - (key % NSW) * SW
        dr_all[c, row, col] = drel
        np.minimum.at(wlo, col, drel)
        np.maximum.at(whi, col, drel)
        np.add.at(cnt_all[c], (w, dl), 1.0)
        cnt_all[c][16, :] = 1.0
        # sort each chunk's rows by source id: SDMA random reads become
        # ascending-address reads (the A matrix absorbs the permutation)
        order = np.argsort(idx_all[c], axis=0, kind="stable")
        idx_all[c] = np.take_along_axis(idx_all[c], order, 0)
        dr_all[c] = np.take_along_axis(dr_all[c], order, 0)

    # chunk plan: (w, sw, ci_base, n_chunks, [(a, wn) per chunk]).
    # Windows tile [0, SW) contiguously: chunk j covers [s_j, e_j) with
    # s_0 = 0, s_j = min-core first-dest, e_j = max(max-core last-dest + 1,
    # s_{j+1}), e_last = SW. Every PSUM column is written by >=1 chunk, so
    # only chunk 0 needs start=True (clears the bank's has_written bits);
    # later chunks overwrite-where-unset / accumulate-where-set.
    plan = []
    for key in range(NKEY):
        cb, n = int(chunk_base[key]), int(nch[key])
        starts = [0] + [int(wlo[cb + j]) for j in range(1, n)]
        ends = [
            max(int(whi[cb + j]) + 1, starts[j + 1]) if j < n - 1 else SW
            for j in range(n)
        ]
        wins = []
        for j in range(n):
            a, wn = starts[j], ends[j] - starts[j]
            assert wn >= 1
            wins.append((a, wn))
            # rebase destrel to the window start (pads stay negative)
            for c in range(NCORES):
                real = dr_all[c, :, cb + j] >= 0
                dr_all[c, real, cb + j] -= a
        plan.append((key // NSW, key % NSW, cb, n, wins))

    inp_pad = np.zeros((cfg.NPAD, D), np.float32)
    inp_pad[: cfg.N] = inp

    # W^T packed [p, 17, kb, o]: slice [:, w, kb, ob*128:(ob+1)*128] is the
    # [K=feat-block, M=out-block] stationary operand.
    Wall = np.concatenate([W_rel, W_self[None]], 0)              # [17, o, in]
    wt = np.ascontiguousarray(
        Wall.transpose(2, 0, 1).reshape(KB, P, 17, D).transpose(1, 2, 0, 3)
    ).astype(BF16)                                               # [p,17,kb,o]

    baug = np.zeros((32, D), np.float32)
    baug[:NW] = b_rel
    baug[16] = b_self
    baug = baug.astype(BF16)

    iota = np.tile(np.arange(SW, dtype=np.float32), (P, 1))

    in_maps = []
    for c in range(NCORES):
        sl = inp_pad[c * CORE_NODES : (c + 1) * CORE_NODES]
        ipt = np.ascontiguousarray(
            sl.T.reshape(KB, P, CORE_NODES).transpose(1, 0, 2)
        ).astype(BF16)                                           # [p, kb, j]
        in_maps.append(
            {
                "inp": inp_pad.astype(BF16),
                "idx": np.ascontiguousarray(idx_all[c]),
                "dr": np.ascontiguousarray(dr_all[c]),
                "iota": iota,
                "ipt": ipt,
                "wt": wt,
                "cnt": cnt_all[c].astype(BF16),
                "baug": baug,
            }
        )
    return in_maps, plan, NCHUNK, int(nch.max())


def _emit(tc, out_ap, ins, cfg, plan, NCHUNK, NCHMAX):
    nc = tc.nc
    f32 = mybir.dt.float32
    bf16 = mybir.dt.bfloat16
    i32 = mybir.dt.int32
    NSW, CORE_NODES = cfg.NSW, cfg.CORE_NODES

    # plan indexed by (w, sw)
    bykey = {}
    for w, sw, cb, n, wins in plan:
        bykey[(w, sw)] = (cb, n, wins)

    with (
        tc.tile_pool(name="const", bufs=1) as const,
        tc.tile_pool(name="xp", bufs=4) as xp,
        tc.tile_pool(name="apl", bufs=10) as apl,
        tc.tile_pool(name="gsb", bufs=5) as gsb,
        tc.tile_pool(name="asb", bufs=2) as asb,
        tc.tile_pool(name="osb", bufs=6) as osb,
        tc.tile_pool(name="gps", bufs=4, space="PSUM") as gps,
        tc.tile_pool(name="aps", bufs=4, space="PSUM") as aps,
    ):
        idx_sb = const.tile([P, NCHUNK], i32)
        nc.sync.dma_start(idx_sb[:], ins["idx"][:, :])
        dr_sb = const.tile([P, NCHUNK], f32)
        nc.sync.dma_start(dr_sb[:], ins["dr"][:, :])
        iota_sb = const.tile([P, SW], f32)
        nc.sync.dma_start(iota_sb[:], ins["iota"][:, :])
        wt_sb = const.tile([P, 17, KB, D], bf16)
        nc.sync.dma_start(wt_sb[:], ins["wt"][:, :, :, :])
        ipt_sb = const.tile([P, KB, CORE_NODES], bf16)
        nc.sync.dma_start(ipt_sb[:], ins["ipt"][:, :, :])
        cnt_sb = const.tile([32, CORE_NODES], bf16)
        nc.sync.dma_start(cnt_sb[:], ins["cnt"][:, :])
        baug_sb = const.tile([32, D], bf16)
        nc.sync.dma_start(baug_sb[:], ins["baug"][:, :])
        inp_dram = ins["inp"]

        for swi in range(NSW):
            # --- agg accumulation, one PSUM bank per 128-dest block:
            # out[dest, feat] directly (G/x/cnt as stationary). bias + self first.
            apt = [aps.tile([P, D], f32, tag="ps", name=f"apt{_db}") for _db in range(KB)]
            for db in range(KB):
                nc.tensor.matmul(
                    apt[db][:],
                    lhsT=cnt_sb[:, ds(swi * SW + db * P, P)],
                    rhs=baug_sb[:, :],
                    start=True,
                    stop=False,
                )
            for kb in range(KB):
                for db in range(KB):
                    nc.tensor.matmul(
                        apt[db][:],
                        lhsT=ipt_sb[:, kb, ds(swi * SW + db * P, P)],
                        rhs=wt_sb[:, 16, kb, :],
                        start=False,
                        stop=False,
                    )

            # software pipeline: agg matmuls run 2 groups behind step2, so the
            # PSUM->SBUF evacuation (DVE) never stalls the in-order PE.
            DELAY = 2
            gts = {}
            for wi in range(NW + DELAY):
                if wi < NW:
                    w = wi
                    cb, n, wins = bykey[(w, swi)]
                    gpt = [gps.tile([P, SW], f32, tag="gp", name=f"gpt{_mb}") for _mb in range(KB)]
                    xt = xp.tile([P, NCHMAX * D], bf16, tag="x")
                    for j in range(n):
                        nc.gpsimd.indirect_dma_start(
                            out=xt[:, j * D : (j + 1) * D],
                            out_offset=None,
                            in_=inp_dram[:, :],
                            in_offset=bass.IndirectOffsetOnAxis(
                                ap=idx_sb[:, cb + j : cb + j + 1], axis=0
                            ),
                        )
                    for j in range(n):
                        a, wn = wins[j]
                        at = apl.tile([P, SW], bf16, tag="a")
                        nc.vector.tensor_scalar(
                            at[:, :wn],
                            iota_sb[:, :wn],
                            dr_sb[:, cb + j : cb + j + 1],
                            None,
                            mybir.AluOpType.is_equal,
                        )
                        for mb in range(KB):
                            nc.tensor.matmul(
                                gpt[mb][:, a : a + wn],
                                lhsT=xt[:, j * D + mb * P : j * D + (mb + 1) * P],
                                rhs=at[:, :wn],
                                start=(j == 0),
                                stop=(j == n - 1),
                            )
                    gt = gsb.tile([P, KB, SW], bf16)
                    for mb in range(KB):
                        nc.vector.tensor_copy(gt[:, mb, :], gpt[mb][:])
                    gts[w] = gt
                if wi >= DELAY:
                    w = wi - DELAY
                    gt = gts.pop(w)
                    for kb in range(KB):
                        for db in range(KB):
                            nc.tensor.matmul(
                                apt[db][:],
                                lhsT=gt[:, kb, ts(db, P)],
                                rhs=wt_sb[:, w, kb, :],
                                start=False,
                                stop=(w == NW - 1 and kb == KB - 1),
                            )

            # --- relu + store (already [dest, feat])
            for db in range(KB):
                ot = osb.tile([P, D], f32)
                nc.vector.tensor_scalar(
                    ot[:], apt[db][:], 0.0, None, mybir.AluOpType.max
                )
                nc.sync.dma_start(out_ap[ds(swi * SW + db * P, P), :], ot[:])


def _build(cfg, plan, NCHUNK, NCHMAX):
    nc = bacc.Bacc("TRN2", target_bir_lowering=False, debug=False,
                   num_devices=NCORES, dynamic_dma_scratch_size=32768)
    f32 = mybir.dt.float32
    ins = {
        "inp": nc.dram_tensor("inp", (cfg.NPAD, D), mybir.dt.bfloat16, kind="ExternalInput").ap(),
        "idx": nc.dram_tensor("idx", (P, NCHUNK), mybir.dt.int32, kind="ExternalInput").ap(),
        "dr": nc.dram_tensor("dr", (P, NCHUNK), f32, kind="ExternalInput").ap(),
        "iota": nc.dram_tensor("iota", (P, SW), f32, kind="ExternalInput").ap(),
        "ipt": nc.dram_tensor("ipt", (P, KB, cfg.CORE_NODES), mybir.dt.bfloat16, kind="ExternalInput").ap(),
        "wt": nc.dram_tensor("wt", (P, 17, KB, D), mybir.dt.bfloat16, kind="ExternalInput").ap(),
        "cnt": nc.dram_tensor("cnt", (32, cfg.CORE_NODES), mybir.dt.bfloat16, kind="ExternalInput").ap(),
        "baug": nc.dram_tensor("baug", (32, D), mybir.dt.bfloat16, kind="ExternalInput").ap(),
    }
    out = nc.dram_tensor("out", (cfg.CORE_NODES, D), f32, kind="ExternalOutput").ap()
    with tile.TileContext(nc) as tc:
        _emit(tc, out, ins, cfg, plan, NCHUNK, NCHMAX)
    nc.compile()
    return nc


def kernel(**inputs):
    global LAST_RESULT
    a = {k: np.asarray(v) for k, v in inputs.items()}
    inp = a["input"].astype(np.float32)
    cfg = make_cfg(inp.shape[0])
    in_maps, plan, NCHUNK, NCHMAX = _host_prep(
        cfg, inp, a["heads"], a["tails"], a["rel"],
        a["W_self"].astype(np.float32), a["b_self"].astype(np.float32),
        a["W_rel"].astype(np.float32), a["b_rel"].astype(np.float32),
    )
    nc = _build(cfg, plan, NCHUNK, NCHMAX)
    res = bass_utils.run_bass_kernel_spmd(
        nc, in_maps, core_ids=list(range(NCORES)),
        trace=bool(os.environ.get("KERNEL_TRACE")),
    )
    LAST_RESULT = res
    full = np.concatenate([res.results[c]["out"] for c in range(NCORES)], 0)
    return full[: cfg.N].astype(np.float32)
